# revision 4
# baseline (speedup 1.0000x reference)
"""Trainium2 Bass kernel for DeepRecurrentModel (encoder MLP + GRU scan + decoder MLP).

Strategy: data-parallel over batch (B=64 -> 8 sequences per core).
All activations kept transposed (feature-major) so the GRU elementwise work
runs across 128 partitions. Matmuls are weight-stationary (orientation
out^T = W^T-tile.T @ x^T) in bf16 with fp32 PSUM accumulation.
The GRU input-side gates GI = e @ Wih^T + bias are precomputed for all
timesteps by the encoder and stored in DRAM scratch (bf16), so the 512-step
sequential scan only does the h @ Whh^T matmul (192 LDW+MM pairs per step)
plus gate elementwise in a dynamic For_i loop.
"""

import numpy as np
import ml_dtypes

import concourse.bass as bass
import concourse.bacc as bacc
import concourse.mybir as mybir
from concourse.tile import TileContext
from concourse import bass_utils

dt = mybir.dt
AF = mybir.ActivationFunctionType

P = 128
B, T_FULL, IN, H, OUT = 64, 512, 128, 1024, 128
NCORES = 8
BL = B // NCORES            # 8 sequences per core
KC = H // P                 # 8 contraction chunks over H
MC_H = H // P               # 8 output chunks over H
MC3 = 3 * H // P            # 24 output chunks over 3H
GW = MC_H * BL              # 64: free width of one gate block (mc-major, b-minor)

_BF16 = np.dtype(ml_dtypes.bfloat16)


def _sb_w(W):
    """[K, M] weight -> SBUF layout [128, (K//128)*M], col = kc*M + m."""
    K, M = W.shape
    kcn = K // P
    return np.ascontiguousarray(
        W.reshape(kcn, P, M).transpose(1, 0, 2).reshape(P, kcn * M)
    )


def _bcol(b):
    """[n*128] bias -> [128, n] per-partition columns."""
    return np.ascontiguousarray(b.reshape(-1, P).T)


def build_program(T=T_FULL, unroll=8):
    """Build the per-core Bass program (SPMD: same program on all cores)."""
    assert T % unroll == 0 and unroll % 2 == 0
    TCW = min(64, T)            # timesteps per encoder token-chunk
    TOKC = TCW * BL             # tokens per chunk (<=512)
    NTC = T // TCW              # number of token chunks
    NXT = TOKC // P             # x tiles of 128 tokens per chunk
    NB = T // unroll            # scan loop bodies
    GF = MC3 * BL               # 192: per-step gi free width

    nc = bacc.Bacc(None, target_bir_lowering=False)

    f32 = dt.float32
    bf16 = dt.bfloat16

    xin = nc.dram_tensor("x_loc", [T * BL, IN], f32, kind="ExternalInput")
    carry = nc.dram_tensor("carry_loc", [BL, H], f32, kind="ExternalInput")
    we0_d = nc.dram_tensor("we0", [P, H], bf16, kind="ExternalInput")
    we1_d = nc.dram_tensor("we1", [P, KC * H], bf16, kind="ExternalInput")
    we2_d = nc.dram_tensor("we2", [P, KC * H], bf16, kind="ExternalInput")
    wih_d = nc.dram_tensor("wihT", [P, KC * 3 * H], bf16, kind="ExternalInput")
    whh_d = nc.dram_tensor("whhT", [P, KC * 3 * H], bf16, kind="ExternalInput")
    wd0_d = nc.dram_tensor("wd0", [P, KC * H], bf16, kind="ExternalInput")
    wd1_d = nc.dram_tensor("wd1", [P, KC * H], bf16, kind="ExternalInput")
    wd2_d = nc.dram_tensor("wd2", [P, KC * OUT], bf16, kind="ExternalInput")
    be0_d = nc.dram_tensor("be0", [P, MC_H], f32, kind="ExternalInput")
    be1_d = nc.dram_tensor("be1", [P, MC_H], f32, kind="ExternalInput")
    be2_d = nc.dram_tensor("be2", [P, MC_H], f32, kind="ExternalInput")
    bgi_d = nc.dram_tensor("bgi", [P, MC3], f32, kind="ExternalInput")
    bn_d = nc.dram_tensor("bn", [P, GW], f32, kind="ExternalInput")
    bd0_d = nc.dram_tensor("bd0", [P, MC_H], f32, kind="ExternalInput")
    bd1_d = nc.dram_tensor("bd1", [P, MC_H], f32, kind="ExternalInput")
    bd2_d = nc.dram_tensor("bd2", [P, 1], f32, kind="ExternalInput")
    id_d = nc.dram_tensor("ident", [P, P], f32, kind="ExternalInput")

    gi_dram = nc.dram_tensor("gi_scratch", [T, P, GF], bf16)

    out_d = nc.dram_tensor("out_loc", [BL, OUT], f32, kind="ExternalOutput")
    hfin_d = nc.dram_tensor("hfin_loc", [BL, H], f32, kind="ExternalOutput")

    with TileContext(nc) as tc:
        with (
            tc.tile_pool(name="wenc", bufs=1) as wp,
            tc.tile_pool(name="consts", bufs=1) as cp,
        ):
            # encoder weights + constants
            we0_t = wp.tile([P, H], bf16)
            we1_t = wp.tile([P, KC * H], bf16)
            we2_t = wp.tile([P, KC * H], bf16)
            wih_t = wp.tile([P, KC * 3 * H], bf16)
            for tgt, src in [(we0_t, we0_d), (we1_t, we1_d), (we2_t, we2_d),
                             (wih_t, wih_d)]:
                nc.sync.dma_start(tgt[:, :], src[:, :])
            btiles = {}
            for nm, src, w in [("be0", be0_d, MC_H), ("be1", be1_d, MC_H),
                               ("be2", be2_d, MC_H), ("bgi", bgi_d, MC3),
                               ("bn", bn_d, GW), ("bd0", bd0_d, MC_H),
                               ("bd1", bd1_d, MC_H), ("bd2", bd2_d, 1)]:
                t = cp.tile([P, w], f32, tag=nm)
                nc.sync.dma_start(t[:, :], src[:, :])
                btiles[nm] = t
            id_t = cp.tile([P, P], f32)
            nc.sync.dma_start(id_t[:, :], id_d[:, :])

            # ---------------- encoder + GI precompute ----------------
            with (
                tc.tile_pool(name="enc", bufs=2) as xe,
                tc.tile_pool(name="enc1", bufs=1) as xe1,
                tc.tile_pool(name="encps", bufs=2, space="PSUM") as eps,
                tc.tile_pool(name="stage", bufs=1) as stg,
            ):
                for tci in range(NTC):
                    xT = xe.tile([P, TOKC], bf16, tag="xT")
                    for i in range(NXT):
                        xs = xe.tile([P, IN], f32, tag="xs")
                        r0 = tci * TOKC + i * P
                        nc.sync.dma_start(xs[:, :], xin[r0:r0 + P, :])
                        tp = eps.tile([P, P], f32, tag="tp")
                        nc.tensor.transpose(tp[:, :], xs[:, :], id_t[:, :])
                        nc.vector.tensor_copy(xT[:, i * P:(i + 1) * P], tp[:, :])

                    e0 = xe1.tile([P, MC_H * TOKC], bf16, tag="e0")
                    for mc in range(MC_H):
                        ps = eps.tile([P, TOKC], f32, tag="mm")
                        nc.tensor.matmul(ps[:, :], we0_t[:, mc * P:(mc + 1) * P],
                                         xT[:, :], start=True, stop=True)
                        nc.scalar.activation(e0[:, mc * TOKC:(mc + 1) * TOKC],
                                             ps[:, :], AF.Relu,
                                             bias=btiles["be0"][:, mc:mc + 1])
                    e1 = xe1.tile([P, MC_H * TOKC], bf16, tag="e1")
                    for mc in range(MC_H):
                        ps = eps.tile([P, TOKC], f32, tag="mm")
                        for kc in range(KC):
                            nc.tensor.matmul(
                                ps[:, :],
                                we1_t[:, kc * H + mc * P: kc * H + (mc + 1) * P],
                                e0[:, kc * TOKC:(kc + 1) * TOKC],
                                start=(kc == 0), stop=(kc == KC - 1))
                        nc.scalar.activation(e1[:, mc * TOKC:(mc + 1) * TOKC],
                                             ps[:, :], AF.Relu,
                                             bias=btiles["be1"][:, mc:mc + 1])
                    e2 = xe1.tile([P, MC_H * TOKC], bf16, tag="e2")
                    for mc in range(MC_H):
                        ps = eps.tile([P, TOKC], f32, tag="mm")
                        for kc in range(KC):
                            nc.tensor.matmul(
                                ps[:, :],
                                we2_t[:, kc * H + mc * P: kc * H + (mc + 1) * P],
                                e1[:, kc * TOKC:(kc + 1) * TOKC],
                                start=(kc == 0), stop=(kc == KC - 1))
                        nc.scalar.activation(e2[:, mc * TOKC:(mc + 1) * TOKC],
                                             ps[:, :], AF.Identity,
                                             bias=btiles["be2"][:, mc:mc + 1])
                    # GI chunk -> staged bf16, strided into per-step layout
                    stage = stg.tile([P, TCW * GF], bf16, tag="st")
                    st3 = stage[:, :].rearrange("p (t f) -> p t f", f=GF)
                    for mc in range(MC3):
                        ps = eps.tile([P, TOKC], f32, tag="mm")
                        for kc in range(KC):
                            nc.tensor.matmul(
                                ps[:, :],
                                wih_t[:, kc * 3 * H + mc * P: kc * 3 * H + (mc + 1) * P],
                                e2[:, kc * TOKC:(kc + 1) * TOKC],
                                start=(kc == 0), stop=(kc == KC - 1))
                        nc.scalar.activation(
                            st3[:, :, mc * BL:(mc + 1) * BL],
                            ps[:, :].rearrange("p (t b) -> p t b", b=BL),
                            AF.Identity, bias=btiles["bgi"][:, mc:mc + 1])
                    nc.sync.dma_start(
                        gi_dram[tci * TCW:(tci + 1) * TCW, :, :]
                        .rearrange("t p f -> p t f"),
                        st3[:, :, :])

            # ---------------- recurrent weights ----------------
            with tc.tile_pool(name="wrec", bufs=1) as wr:
                whh_t = wr.tile([P, KC * 3 * H], bf16)
                nc.sync.dma_start(whh_t[:, :], whh_d[:, :])

                with (
                    tc.tile_pool(name="hstate", bufs=1) as hp,
                    tc.tile_pool(name="scratch", bufs=2) as sp,
                ):
                    hf = [hp.tile([P, GW], f32, tag=f"hf{j}", name=f"hf{j}") for j in range(2)]
                    hb = [hp.tile([P, GW], bf16, tag=f"hb{j}", name=f"hb{j}") for j in range(2)]
                    gi_tiles = [hp.tile([P, GF], bf16, tag=f"gi{u}",
                                         name=f"gi{u}") for u in range(unroll)]
                    sps_ctx = tc.tile_pool(name="scanps", bufs=2, space="PSUM")
                    sps = sps_ctx.__enter__()

                    # h0: transpose carry [BL, H] -> hT [128, kc*BL]
                    cs = sp.tile([BL, H], f32, tag="carry")
                    nc.sync.dma_start(cs[:, :], carry[:, :])
                    for kc in range(KC):
                        tp = sps.tile([P, BL], f32, tag="h0t")
                        nc.tensor.transpose(tp[:, :], cs[:, kc * P:(kc + 1) * P],
                                            id_t[:BL, :BL])
                        nc.vector.tensor_copy(hf[0][:, kc * BL:(kc + 1) * BL],
                                              tp[:, :])
                        nc.scalar.copy(hb[0][:, kc * BL:(kc + 1) * BL], tp[:, :])

                    gi_v = gi_dram[:, :, :].rearrange("t p f -> p t f")

                    # ---------------- GRU scan ----------------
                    with tc.For_i(0, NB, 1) as ib:
                        for u in range(unroll):
                            nc.sync.dma_start(
                                gi_tiles[u][:, :].rearrange(
                                    "p (a f) -> p a f", a=1),
                                gi_v[:, bass.ds(ib * unroll + u, 1), :])
                        for u in range(unroll):
                            cur, nxt = u % 2, (u + 1) % 2
                            h_in_b, h_in_f = hb[cur], hf[cur]
                            h_out_b, h_out_f = hb[nxt], hf[nxt]
                            gi = gi_tiles[u]
                            ps_g = [sps.tile([P, GW], f32, tag=f"ps{g}",
                                              name=f"ps{g}_{u}") for g in range(3)]
                            for g in (0, 2, 1):  # r, n, z (z last: frees tail)
                                ps = ps_g[g]
                                for mcl in range(MC_H):
                                    mcg = g * MC_H + mcl
                                    for kc in range(KC):
                                        nc.tensor.matmul(
                                            ps[:, mcl * BL:(mcl + 1) * BL],
                                            whh_t[:, kc * 3 * H + mcg * P:
                                                  kc * 3 * H + (mcg + 1) * P],
                                            h_in_b[:, kc * BL:(kc + 1) * BL],
                                            start=(kc == 0), stop=(kc == KC - 1))
                            rpre = sp.tile([P, GW], f32, tag="rpre")
                            nc.vector.tensor_add(rpre[:, :], ps_g[0][:, :],
                                                 gi[:, 0:GW])
                            r_s = sp.tile([P, GW], f32, tag="r_s")
                            nc.scalar.activation(r_s[:, :], rpre[:, :], AF.Sigmoid)
                            hnb = sp.tile([P, GW], f32, tag="hnb")
                            nc.vector.tensor_add(hnb[:, :], ps_g[2][:, :],
                                                 btiles["bn"][:, :])
                            t1 = sp.tile([P, GW], f32, tag="t1")
                            nc.vector.tensor_mul(t1[:, :], r_s[:, :], hnb[:, :])
                            t2 = sp.tile([P, GW], f32, tag="t2")
                            nc.vector.tensor_add(t2[:, :], t1[:, :],
                                                 gi[:, 2 * GW:3 * GW])
                            n_t = sp.tile([P, GW], f32, tag="n_t")
                            nc.scalar.activation(n_t[:, :], t2[:, :], AF.Tanh)
                            dmn = sp.tile([P, GW], f32, tag="dmn")
                            nc.vector.tensor_sub(dmn[:, :], h_in_f[:, :],
                                                 n_t[:, :])
                            zpre = sp.tile([P, GW], f32, tag="zpre")
                            nc.vector.tensor_add(zpre[:, :], ps_g[1][:, :],
                                                 gi[:, GW:2 * GW])
                            z_s = sp.tile([P, GW], f32, tag="z_s")
                            nc.scalar.activation(z_s[:, :], zpre[:, :], AF.Sigmoid)
                            e_t = sp.tile([P, GW], f32, tag="e_t")
                            nc.vector.tensor_mul(e_t[:, :], z_s[:, :], dmn[:, :])
                            nc.vector.tensor_add(h_out_f[:, :], n_t[:, :],
                                                 e_t[:, :])
                            nc.scalar.copy(h_out_b[:, :], h_out_f[:, :])

                    # ---------------- decoder (on final h) ----------------
                    sps_ctx.__exit__(None, None, None)
                    with (
                        tc.tile_pool(name="wdec", bufs=1) as wd,
                        tc.tile_pool(name="decps", bufs=2, space="PSUM") as dps,
                    ):
                        wd0_t = wd.tile([P, KC * H], bf16)
                        wd1_t = wd.tile([P, KC * H], bf16)
                        wd2_t = wd.tile([P, KC * OUT], bf16)
                        for tgt, src in [(wd0_t, wd0_d), (wd1_t, wd1_d),
                                         (wd2_t, wd2_d)]:
                            nc.sync.dma_start(tgt[:, :], src[:, :])

                        def dec_layer(src_b, w_t, bias, func, width):
                            dst = wd.tile([P, width * BL], bf16,
                                          tag=f"dec{id(w_t)}")
                            for mc in range(width):
                                ps = dps.tile([P, BL], f32, tag="dmm")
                                for kc in range(KC):
                                    nc.tensor.matmul(
                                        ps[:, :],
                                        w_t[:, kc * width * P + mc * P:
                                            kc * width * P + (mc + 1) * P],
                                        src_b[:, kc * BL:(kc + 1) * BL],
                                        start=(kc == 0), stop=(kc == KC - 1))
                                nc.scalar.activation(
                                    dst[:, mc * BL:(mc + 1) * BL], ps[:, :],
                                    func, bias=bias[:, mc:mc + 1])
                            return dst

                        d0 = dec_layer(hb[0], wd0_t, btiles["bd0"], AF.Relu, MC_H)
                        d1 = dec_layer(d0, wd1_t, btiles["bd1"], AF.Relu, MC_H)
                        # out layer: OUT=128 -> single mc
                        pso = dps.tile([P, BL], f32, tag="dmm")
                        for kc in range(KC):
                            nc.tensor.matmul(
                                pso[:, :],
                                wd2_t[:, kc * OUT:(kc + 1) * OUT],
                                d1[:, kc * BL:(kc + 1) * BL],
                                start=(kc == 0), stop=(kc == KC - 1))
                        outT = sp.tile([P, BL], f32, tag="outT")
                        nc.scalar.activation(outT[:, :], pso[:, :], AF.Identity,
                                             bias=btiles["bd2"][:, 0:1])

                        # transpose back to natural layout + store
                        onat = sp.tile([BL, OUT], f32, tag="onat")
                        tpo = dps.tile([BL, P], f32, tag="tpo")
                        nc.tensor.transpose(tpo[:, :], outT[:, :], id_t[:, :])
                        nc.vector.tensor_copy(onat[:, :], tpo[:, :])
                        nc.sync.dma_start(out_d[:, :], onat[:, :])

                        hnat = sp.tile([BL, H], f32, tag="hnat")
                        for kc in range(KC):
                            tph = dps.tile([BL, P], f32, tag="tpo")
                            nc.tensor.transpose(tph[:, :],
                                                hf[0][:, kc * BL:(kc + 1) * BL],
                                                id_t[:, :])
                            nc.vector.tensor_copy(hnat[:, kc * P:(kc + 1) * P],
                                                  tph[:, :])
                        nc.sync.dma_start(hfin_d[:, :], hnat[:, :])

    nc.compile()
    return nc


_CACHE = {}


def _get_program(T=T_FULL, unroll=8):
    key = (T, unroll)
    if key not in _CACHE:
        _CACHE[key] = build_program(T, unroll)
    return _CACHE[key]


def prep_host_inputs(inputs, T=T_FULL):
    """Fold normalization, transpose/relayout weights, build per-core maps."""
    f = {k: np.asarray(v, np.float32) for k, v in inputs.items()}
    std = f["std"]; mean = f["mean"]
    We0p = f["We0"] / std[:, None]
    be0p = f["be0"] - (mean / std) @ f["We0"]
    bias_gi = f["bih"].copy()
    bias_gi[:2 * H] += f["bhh"][:2 * H]
    bhh_n = f["bhh"][2 * H:]

    def bfw(a):
        return np.ascontiguousarray(a).astype(_BF16)

    shared = {
        "we0": bfw(We0p),
        "we1": bfw(_sb_w(f["We1"])),
        "we2": bfw(_sb_w(f["We2"])),
        "wihT": bfw(_sb_w(np.ascontiguousarray(f["Wih"].T))),
        "whhT": bfw(_sb_w(np.ascontiguousarray(f["Whh"].T))),
        "wd0": bfw(_sb_w(f["Wd0"])),
        "wd1": bfw(_sb_w(f["Wd1"])),
        "wd2": bfw(_sb_w(f["Wd2"])),
        "be0": _bcol(be0p), "be1": _bcol(f["be1"]), "be2": _bcol(f["be2"]),
        "bgi": _bcol(bias_gi),
        "bn": np.ascontiguousarray(
            np.repeat(bhh_n.reshape(MC_H, P).T[:, :, None], BL, axis=2)
            .reshape(P, GW)),
        "bd0": _bcol(f["bd0"]), "bd1": _bcol(f["bd1"]),
        "bd2": _bcol(f["bd2"]),
        "ident": np.eye(P, dtype=np.float32),
    }
    in_maps = []
    x = f["x"][:, :T, :]
    carry = f["carry"]
    for c in range(NCORES):
        xc = np.ascontiguousarray(
            x[c * BL:(c + 1) * BL].transpose(1, 0, 2).reshape(T * BL, IN))
        cc = np.ascontiguousarray(carry[c * BL:(c + 1) * BL, 0, :])
        m = dict(shared)
        m["x_loc"] = xc
        m["carry_loc"] = cc
        in_maps.append(m)
    return in_maps


def kernel(**inputs):
    nc = _get_program()
    in_maps = prep_host_inputs(inputs)
    res = bass_utils.run_bass_kernel_spmd(nc, in_maps,
                                          core_ids=list(range(NCORES)))
    out = np.empty((B, 1, OUT), np.float32)
    hfin = np.empty((B, 1, H), np.float32)
    for c in range(NCORES):
        out[c * BL:(c + 1) * BL, 0, :] = res.results[c]["out_loc"]
        hfin[c * BL:(c + 1) * BL, 0, :] = res.results[c]["hfin_loc"]
    return out, hfin
